# revision 4
# baseline (speedup 1.0000x reference)
"""DistMult decoder kernel for 8 Trainium2 NeuronCores.

Computes out = (input1 * weight[type_index]) @ input2.T + bias with
input1 [8192, 512], input2 [8192, 512] in fp32, out [8192, 8192].

Sharding: rows of input1 (and thus rows of the output) are split across
the 8 cores; input2 / weight / bias are replicated. No communication.

Mixed-precision column split: the per-column quantization error of
both GEMM operands is amplified by |w_r[j]|, so the 256 k-columns with
the largest |w_r| run in fp16 (1 cycle/row) and the 256 smallest run in
fp8-e4m3 using the PE's DoubleRow mode (2 k-tiles per instruction, 2
rows/cycle).  Measured rel-err vs the fp32 reference: 1.14e-2 (gate
2e-2).  PE stream: 8n x 8m x (2 fp16 @512 rows + 4 DR @256 rows) =
196608 cycles ~ 82 us, vs 262144 ~ 109 us for all-fp16.

The fp16<->fp8 PE mode switch costs ~200 ns (first DR matmul after the
switch pays an unhidden 256-row LDWEIGHTS + pipeline bubble), so m-tiles
are processed in pairs: 8 fp16 matmuls for (m, m+1), then 8 DR matmuls,
halving the number of switches.

The output is stored as fp16 (upcast to fp32 on host), halving store
traffic: 22.75 MB/core total vs 41 MB for the fp32-out baseline.
"""

import os

import numpy as np
import ml_dtypes

import concourse.bacc as bacc
import concourse.mybir as mybir
from concourse.bass_utils import run_bass_kernel_spmd
from concourse.tile import TileContext

N_CORES = 8
N1, N2, D = 8192, 8192, 512
M = N1 // N_CORES  # rows per core
P = 128            # partitions
DH = 256           # hi (fp16) k-columns
DL = 256           # lo (fp8) k-columns
KH = DH // P       # 2 fp16 k-tiles
NFREE = 512        # psum bank free size (fp32)
NGRP = 1024        # n columns per group (pair of psum banks)
NT = N2 // NGRP    # 8 n-groups
MT = M // P        # 8 m-tiles

# test.py hooks: set TRACE=True before calling kernel() to profile; the
# BassKernelResults of the last run lands in LAST_RESULTS.
TRACE = os.environ.get("BASS_KERNEL_TRACE", "0") == "1"
LAST_RESULTS = None

_cached_nc = None


def _build():
    nc = bacc.Bacc(
        "TRN2", target_bir_lowering=False, debug=False, enable_asserts=False, num_devices=N_CORES
    )
    f32 = mybir.dt.float32
    f16 = mybir.dt.float16
    f8 = mybir.dt.float8e4
    DR = mybir.MatmulPerfMode.DoubleRow

    lhsTH = nc.dram_tensor("lhsTH", [DH, M], f16, kind="ExternalInput")
    lhsTL = nc.dram_tensor("lhsTL", [DL, M], f8, kind="ExternalInput")
    rhsH = nc.dram_tensor("rhsH", [DH, N2], f16, kind="ExternalInput")
    rhsL = nc.dram_tensor("rhsL", [DL, N2], f8, kind="ExternalInput")
    biasv = nc.dram_tensor("biasv", [P, 1], f32, kind="ExternalInput")
    out = nc.dram_tensor("out", [M, N2], f16, kind="ExternalOutput")

    # K-major DRAM views split into [P, kt, cols] for single-DMA loads.
    lhsTH_r = lhsTH[:, :].rearrange("(kt p) m -> p kt m", p=P)
    lhsTL_r = lhsTL[:, :].rearrange("(kt p) m -> p kt m", p=P)
    rhsH_r = rhsH[:, :].rearrange("(kt p) n -> p kt n", p=P)
    rhsL_r = rhsL[:, :].rearrange("(kt p) n -> p kt n", p=P)

    with TileContext(nc) as tc:
        with (
            tc.tile_pool(name="const", bufs=1) as constp,
            tc.tile_pool(name="lhs", bufs=1) as lhsp,
            tc.tile_pool(name="rhsp", bufs=3) as rhsp,
            tc.tile_pool(name="outp", bufs=8) as outp,
            tc.tile_pool(name="psum", bufs=2, space="PSUM") as psump,
        ):
            # Head: spread the startup loads across all three DGE rings
            # (each ring tops out well below HBM bandwidth) so the PE can
            # start as soon as the preamble ends.  The first fp16 matmuls
            # need ltH k0 + rtH0 k0; those go first on separate rings.
            ltH = lhsp.tile([P, KH, M], f16, tag="lhsH")
            ltL = lhsp.tile([P, KH, M], f8, tag="lhsL")
            rtH0 = rhsp.tile([P, KH, NGRP], f16, tag="rhsH")
            rtL0 = rhsp.tile([P, KH, NGRP], f8, tag="rhsL")
            nc.sync.dma_start(out=rtH0[:, 0, :], in_=rhsH_r[:, 0, 0:NGRP])
            nc.scalar.dma_start(out=ltH[:, 0, :], in_=lhsTH_r[:, 0, :])
            nc.gpsimd.dma_start(out=rtH0[:, 1, :], in_=rhsH_r[:, 1, 0:NGRP])
            nc.scalar.dma_start(out=ltH[:, 1, :], in_=lhsTH_r[:, 1, :])
            nc.sync.dma_start(out=ltL[:], in_=lhsTL_r[:, :, :])
            bias_t = constp.tile([P, 1], f32, tag="bias")
            nc.scalar.dma_start(out=bias_t[:], in_=biasv[:, :])
            nc.gpsimd.dma_start(out=rtL0[:], in_=rhsL_r[:, :, 0:NGRP])
            # Group 1 rides the HWDGE rings during the head (they idle
            # until the first stores ~4.5 us in); gpsimd handles groups 2+
            # with two groups of lookahead.
            rtH1 = rhsp.tile([P, KH, NGRP], f16, tag="rhsH")
            rtL1 = rhsp.tile([P, KH, NGRP], f8, tag="rhsL")
            nc.sync.dma_start(out=rtH1[:], in_=rhsH_r[:, :, NGRP : 2 * NGRP])
            nc.scalar.dma_start(out=rtL1[:], in_=rhsL_r[:, :, NGRP : 2 * NGRP])

            # Warm up the PE's HAM clock gate during the head-load window:
            # dummy matmuls on zeroed SBUF (no data deps) push the PE
            # through its busy window so the real matmuls start at 2.4 GHz
            # instead of ramping from 1.2 GHz.
            warm_w = constp.tile([P, P], f16, tag="warmw")
            warm_r = constp.tile([P, NFREE], f16, tag="warmr")
            nc.vector.memset(warm_w[:], 0.0)
            nc.vector.memset(warm_r[:], 0.0)
            wps = psump.tile([P, NFREE], f32, tag="ps1")
            NWARM = 12
            for i in range(NWARM):
                nc.tensor.matmul(
                    wps[:], warm_w[:], warm_r[:],
                    start=(i == 0), stop=(i == NWARM - 1),
                )

            rts = {0: (rtH0, rtL0), 1: (rtH1, rtL1)}

            def load_rhs(g):
                rtH = rhsp.tile([P, KH, NGRP], f16, tag="rhsH")
                rtL = rhsp.tile([P, KH, NGRP], f8, tag="rhsL")
                nc.gpsimd.dma_start(
                    out=rtH[:], in_=rhsH_r[:, :, g * NGRP : (g + 1) * NGRP]
                )
                nc.gpsimd.dma_start(
                    out=rtL[:], in_=rhsL_r[:, :, g * NGRP : (g + 1) * NGRP]
                )
                rts[g] = (rtH, rtL)

            for n in range(NT):
                rtH, rtL = rts.pop(n)
                for mp in range(MT // 2):  # m-tile pairs
                    if mp == 0 and n + 2 < NT:
                        load_rhs(n + 2)
                    pss = []
                    # fp16 hi columns for both m-tiles of the pair, then
                    # all fp8 DoubleRow matmuls: one PE mode switch per
                    # pair instead of two per m-tile.
                    for mi in range(2):
                        m = 2 * mp + mi
                        ms = slice(m * P, (m + 1) * P)
                        ps0 = psump.tile([P, NFREE], f32, tag=f"ps{2 * mi}")
                        ps1 = psump.tile([P, NFREE], f32, tag=f"ps{2 * mi + 1}")
                        pss.append((ps0, ps1, m, ms))
                        nc.tensor.matmul(
                            ps0[:], ltH[:, 0, ms], rtH[:, 0, 0:NFREE],
                            start=True, stop=False,
                        )
                        nc.tensor.matmul(
                            ps1[:], ltH[:, 0, ms], rtH[:, 0, NFREE:NGRP],
                            start=True, stop=False,
                        )
                        nc.tensor.matmul(
                            ps0[:], ltH[:, 1, ms], rtH[:, 1, 0:NFREE],
                            start=False, stop=False,
                        )
                        nc.tensor.matmul(
                            ps1[:], ltH[:, 1, ms], rtH[:, 1, NFREE:NGRP],
                            start=False, stop=False,
                        )
                    for ps0, ps1, m, ms in pss:
                        nc.tensor.matmul(
                            ps0[:, 0:256], ltL[:, :, ms], rtL[:, :, 0:256],
                            start=False, stop=True, perf_mode=DR,
                            skip_group_check=True,
                        )
                        nc.tensor.matmul(
                            ps0[:, 256:512], ltL[:, :, ms], rtL[:, :, 256:512],
                            start=False, stop=True, perf_mode=DR,
                            skip_group_check=True,
                        )
                        nc.tensor.matmul(
                            ps1[:, 0:256], ltL[:, :, ms], rtL[:, :, 512:768],
                            start=False, stop=True, perf_mode=DR,
                            skip_group_check=True,
                        )
                        nc.tensor.matmul(
                            ps1[:, 256:512], ltL[:, :, ms], rtL[:, :, 768:1024],
                            start=False, stop=True, perf_mode=DR,
                            skip_group_check=True,
                        )
                    for ps0, ps1, m, ms in pss:
                        ot = outp.tile([P, NGRP], f16, tag="ot")
                        # Split psum->sbuf+bias between ACT and the
                        # otherwise idle DVE so neither serializes the psum
                        # pool; both downcast to fp16 on the way out.
                        nc.scalar.activation(
                            ot[:, 0:NFREE], ps0[:],
                            mybir.ActivationFunctionType.Identity,
                            bias=bias_t[:, 0:1],
                        )
                        nc.vector.tensor_scalar_add(
                            ot[:, NFREE:NGRP], ps1[:], bias_t[:, 0:1]
                        )
                        if n == NT - 1 and m == MT - 1:
                            # Final tile: store in quarters on both rings so
                            # the kernel-exit barrier isn't waiting on one
                            # serial copy+store chain.
                            for q, eng in enumerate(
                                (nc.sync, nc.scalar, nc.sync, nc.scalar)
                            ):
                                eng.dma_start(
                                    out=out[m * P : (m + 1) * P,
                                            n * NGRP + q * 256
                                            : n * NGRP + (q + 1) * 256],
                                    in_=ot[:, q * 256 : (q + 1) * 256],
                                )
                        else:
                            # Alternate stores across the two HWDGE rings so
                            # the store stream drains on both.
                            st = nc.sync if m % 2 == 0 else nc.scalar
                            st.dma_start(
                                out=out[m * P : (m + 1) * P,
                                        n * NGRP : (n + 1) * NGRP],
                                in_=ot[:],
                            )
    nc.compile()
    return nc


def kernel(input1, input2, weight, bias, type_index):
    global _cached_nc, LAST_RESULTS

    input1 = np.asarray(input1, dtype=np.float32)
    input2 = np.asarray(input2, dtype=np.float32)
    weight = np.asarray(weight, dtype=np.float32)
    bias = np.asarray(bias, dtype=np.float32).reshape(-1)
    w_r = weight[int(type_index)]  # [D]

    # Host-side prep: fold the w_r row-scale into input1, split k-columns
    # by |w_r| (largest -> fp16, smallest -> fp8), lay both GEMM operands
    # out K-major (device accumulates in fp32).
    order = np.argsort(-np.abs(w_r), kind="stable")
    hi = np.sort(order[:DH])
    lo = np.sort(order[DH:])
    f8 = ml_dtypes.float8_e4m3
    scaled = input1 * w_r[None, :]  # [N1, D]
    rhsH = np.ascontiguousarray(input2[:, hi].T).astype(np.float16)  # [DH, N2]
    rhsL = np.ascontiguousarray(input2[:, lo].T).astype(f8)          # [DL, N2]
    bias_vec = np.full((P, 1), float(bias[0]), dtype=np.float32)

    scaledH = scaled[:, hi]
    scaledL = scaled[:, lo]
    in_maps = []
    for c in range(N_CORES):
        sl = slice(c * M, (c + 1) * M)
        in_maps.append(
            {
                "lhsTH": np.ascontiguousarray(scaledH[sl].T).astype(np.float16),
                "lhsTL": np.ascontiguousarray(scaledL[sl].T).astype(f8),
                "rhsH": rhsH,
                "rhsL": rhsL,
                "biasv": bias_vec,
            }
        )

    if _cached_nc is None:
        _cached_nc = _build()

    res = run_bass_kernel_spmd(
        _cached_nc, in_maps, core_ids=list(range(N_CORES)), trace=TRACE
    )
    LAST_RESULTS = res
    return np.concatenate(
        [res.results[c]["out"] for c in range(N_CORES)], axis=0
    ).astype(np.float32)


# revision 9
# speedup vs baseline: 1.0311x; 1.0311x over previous
"""DistMult decoder kernel for 8 Trainium2 NeuronCores.

Computes out = (input1 * weight[type_index]) @ input2.T + bias with
input1 [8192, 512], input2 [8192, 512] in fp32, out [8192, 8192].

Sharding: rows of input1 (and thus rows of the output) are split across
the 8 cores; input2 / weight / bias are replicated. No communication.

Mixed-precision column split: the per-column quantization error of
both GEMM operands is amplified by |w_r[j]|, so the 256 k-columns with
the largest |w_r| run in fp16 (1 cycle/row) and the 256 smallest run in
fp8-e4m3 using the PE's DoubleRow mode (2 k-tiles per instruction, 2
rows/cycle).  Measured rel-err vs the fp32 reference: 1.14e-2 (gate
2e-2).  PE stream: 8n x 8m x (2 fp16 @512 rows + 4 DR @256 rows) =
196608 cycles ~ 82 us, vs 262144 ~ 109 us for all-fp16.

The fp16<->fp8 PE mode switch costs ~200 ns (first DR matmul after the
switch pays an unhidden 256-row LDWEIGHTS + pipeline bubble), so m-tiles
are processed in pairs: 8 fp16 matmuls for (m, m+1), then 8 DR matmuls,
halving the number of switches.

The output is stored as fp16 (upcast to fp32 on host), halving store
traffic: 22.75 MB/core total vs 41 MB for the fp32-out baseline.
"""

import os

import numpy as np
import ml_dtypes

import concourse.bacc as bacc
import concourse.mybir as mybir
from concourse.bass_utils import run_bass_kernel_spmd
from concourse.tile import TileContext

N_CORES = 8
N1, N2, D = 8192, 8192, 512
M = N1 // N_CORES  # rows per core
P = 128            # partitions
DH = 256           # hi (fp16) k-columns
DL = 256           # lo (fp8) k-columns
KH = DH // P       # 2 fp16 k-tiles
NFREE = 512        # psum bank free size (fp32)
NGRP = 1024        # n columns per group (pair of psum banks)
NT = N2 // NGRP    # 8 n-groups
MT = M // P        # 8 m-tiles

# test.py hooks: set TRACE=True before calling kernel() to profile; the
# BassKernelResults of the last run lands in LAST_RESULTS.
TRACE = os.environ.get("BASS_KERNEL_TRACE", "0") == "1"
LAST_RESULTS = None

_cached_nc = None


def _build():
    nc = bacc.Bacc(
        "TRN2", target_bir_lowering=False, debug=False, enable_asserts=False, num_devices=N_CORES
    )
    f32 = mybir.dt.float32
    f16 = mybir.dt.float16
    f8 = mybir.dt.float8e4
    DR = mybir.MatmulPerfMode.DoubleRow

    lhsTH = nc.dram_tensor("lhsTH", [DH, M], f16, kind="ExternalInput")
    lhsTL = nc.dram_tensor("lhsTL", [DL, M], f8, kind="ExternalInput")
    rhsH = nc.dram_tensor("rhsH", [DH, N2], f16, kind="ExternalInput")
    rhsL = nc.dram_tensor("rhsL", [DL, N2], f8, kind="ExternalInput")
    biasv = nc.dram_tensor("biasv", [P, 1], f32, kind="ExternalInput")
    out = nc.dram_tensor("out", [M, N2], f16, kind="ExternalOutput")

    # K-major DRAM views split into [P, kt, cols] for single-DMA loads.
    lhsTH_r = lhsTH[:, :].rearrange("(kt p) m -> p kt m", p=P)
    lhsTL_r = lhsTL[:, :].rearrange("(kt p) m -> p kt m", p=P)
    rhsH_r = rhsH[:, :].rearrange("(kt p) n -> p kt n", p=P)
    rhsL_r = rhsL[:, :].rearrange("(kt p) n -> p kt n", p=P)

    with TileContext(nc) as tc:
        with (
            tc.tile_pool(name="const", bufs=1) as constp,
            tc.tile_pool(name="lhs", bufs=1) as lhsp,
            tc.tile_pool(name="rhsp", bufs=2) as rhsp,
            tc.tile_pool(name="outp", bufs=8) as outp,
            tc.tile_pool(name="psum", bufs=2, space="PSUM") as psump,
        ):
            # Head: spread the startup loads across all three DGE rings
            # (each ring tops out well below HBM bandwidth) so the PE can
            # start as soon as the preamble ends.  The first fp16 matmuls
            # need ltH k0 + rtH0 k0; those go first on separate rings.
            # Data DMA cannot start before ~8.3 us (fixed runtime startup)
            # and the shared DMA fabric serves issued descriptors roughly
            # in order at ~290 GB/s aggregate, so the six pieces the first
            # m-pair consumes go on the two HWDGE rings in consumption
            # order (SWDGE starts ~2 us later and is 1.6x slower — only
            # group 1, needed at ~21 us, rides it).
            ltH = lhsp.tile([P, KH, M], f16, tag="lhsH")
            ltL = lhsp.tile([P, KH, M], f8, tag="lhsL")
            rtH0 = rhsp.tile([P, KH, NGRP], f16, tag="rhsH")
            rtL0 = rhsp.tile([P, KH, NGRP], f8, tag="rhsL")
            nc.sync.dma_start(out=rtH0[:, 0, :], in_=rhsH_r[:, 0, 0:NGRP])
            nc.scalar.dma_start(out=ltH[:, 0, :], in_=lhsTH_r[:, 0, :])
            nc.sync.dma_start(out=rtH0[:, 1, :], in_=rhsH_r[:, 1, 0:NGRP])
            nc.scalar.dma_start(out=ltH[:, 1, :], in_=lhsTH_r[:, 1, :])
            nc.sync.dma_start(out=rtL0[:], in_=rhsL_r[:, :, 0:NGRP])
            nc.scalar.dma_start(out=ltL[:], in_=lhsTL_r[:, :, :])
            bias_t = constp.tile([P, 1], f32, tag="bias")
            nc.scalar.dma_start(out=bias_t[:], in_=biasv[:, :])
            # Group 1 rides the tails of the same HWDGE rings: ring FIFO
            # order keeps it from competing with the critical set above.
            rtH1 = rhsp.tile([P, KH, NGRP], f16, tag="rhsH")
            rtL1 = rhsp.tile([P, KH, NGRP], f8, tag="rhsL")
            nc.sync.dma_start(out=rtH1[:], in_=rhsH_r[:, :, NGRP : 2 * NGRP])
            nc.scalar.dma_start(out=rtL1[:], in_=rhsL_r[:, :, NGRP : 2 * NGRP])

            # Warm up the PE's HAM clock gate during the head-load window:
            # dummy matmuls on zeroed SBUF (no data deps) push the PE
            # through its busy window so the real matmuls start at 2.4 GHz
            # instead of ramping from 1.2 GHz.
            warm_w = constp.tile([P, P], f16, tag="warmw")
            warm_r = constp.tile([P, NFREE], f16, tag="warmr")
            nc.vector.memset(warm_w[:], 0.0)
            nc.vector.memset(warm_r[:], 0.0)
            # 18 iterations: busy until ~10 us, when the first loads land.
            wps = psump.tile([P, NFREE], f32, tag="ps1")
            NWARM = 18
            for i in range(NWARM):
                nc.tensor.matmul(
                    wps[:], warm_w[:], warm_r[:],
                    start=(i == 0), stop=(i == NWARM - 1),
                )

            rts = {0: (rtH0, rtL0), 1: (rtH1, rtL1)}

            def load_rhs(g):
                rtH = rhsp.tile([P, KH, NGRP], f16, tag="rhsH")
                rtL = rhsp.tile([P, KH, NGRP], f8, tag="rhsL")
                nc.gpsimd.dma_start(
                    out=rtH[:], in_=rhsH_r[:, :, g * NGRP : (g + 1) * NGRP]
                )
                nc.gpsimd.dma_start(
                    out=rtL[:], in_=rhsL_r[:, :, g * NGRP : (g + 1) * NGRP]
                )
                rts[g] = (rtH, rtL)

            for n in range(NT):
                rtH, rtL = rts.pop(n)
                for mp in range(MT // 2):  # m-tile pairs
                    if mp == 0 and n + 2 < NT:
                        load_rhs(n + 2)
                    pss = []
                    # fp16 hi columns for both m-tiles of the pair, then
                    # all fp8 DoubleRow matmuls: one PE mode switch per
                    # pair instead of two per m-tile.
                    for mi in range(2):
                        m = 2 * mp + mi
                        ms = slice(m * P, (m + 1) * P)
                        ps0 = psump.tile([P, NFREE], f32, tag=f"ps{2 * mi}")
                        ps1 = psump.tile([P, NFREE], f32, tag=f"ps{2 * mi + 1}")
                        pss.append((ps0, ps1, m, ms))
                        nc.tensor.matmul(
                            ps0[:], ltH[:, 0, ms], rtH[:, 0, 0:NFREE],
                            start=True, stop=False,
                        )
                        nc.tensor.matmul(
                            ps1[:], ltH[:, 0, ms], rtH[:, 0, NFREE:NGRP],
                            start=True, stop=False,
                        )
                        nc.tensor.matmul(
                            ps0[:], ltH[:, 1, ms], rtH[:, 1, 0:NFREE],
                            start=False, stop=False,
                        )
                        nc.tensor.matmul(
                            ps1[:], ltH[:, 1, ms], rtH[:, 1, NFREE:NGRP],
                            start=False, stop=False,
                        )
                    for ps0, ps1, m, ms in pss:
                        nc.tensor.matmul(
                            ps0[:, 0:256], ltL[:, :, ms], rtL[:, :, 0:256],
                            start=False, stop=True, perf_mode=DR,
                            skip_group_check=True,
                        )
                        nc.tensor.matmul(
                            ps0[:, 256:512], ltL[:, :, ms], rtL[:, :, 256:512],
                            start=False, stop=True, perf_mode=DR,
                            skip_group_check=True,
                        )
                        nc.tensor.matmul(
                            ps1[:, 0:256], ltL[:, :, ms], rtL[:, :, 512:768],
                            start=False, stop=True, perf_mode=DR,
                            skip_group_check=True,
                        )
                        nc.tensor.matmul(
                            ps1[:, 256:512], ltL[:, :, ms], rtL[:, :, 768:1024],
                            start=False, stop=True, perf_mode=DR,
                            skip_group_check=True,
                        )
                    for ps0, ps1, m, ms in pss:
                        ot = outp.tile([P, NGRP], f16, tag="ot")
                        # Split psum->sbuf+bias between ACT and the
                        # otherwise idle DVE so neither serializes the psum
                        # pool; both downcast to fp16 on the way out.
                        nc.scalar.activation(
                            ot[:, 0:NFREE], ps0[:],
                            mybir.ActivationFunctionType.Identity,
                            bias=bias_t[:, 0:1],
                        )
                        nc.vector.tensor_scalar_add(
                            ot[:, NFREE:NGRP], ps1[:], bias_t[:, 0:1]
                        )
                        if n == NT - 1 and m == MT - 1:
                            # Final tile: store in quarters on both rings so
                            # the kernel-exit barrier isn't waiting on one
                            # serial copy+store chain.
                            for q, eng in enumerate(
                                (nc.sync, nc.scalar, nc.sync, nc.scalar)
                            ):
                                eng.dma_start(
                                    out=out[m * P : (m + 1) * P,
                                            n * NGRP + q * 256
                                            : n * NGRP + (q + 1) * 256],
                                    in_=ot[:, q * 256 : (q + 1) * 256],
                                )
                        else:
                            # Alternate stores across the two HWDGE rings so
                            # the store stream drains on both.
                            st = nc.sync if m % 2 == 0 else nc.scalar
                            st.dma_start(
                                out=out[m * P : (m + 1) * P,
                                        n * NGRP : (n + 1) * NGRP],
                                in_=ot[:],
                            )
    nc.compile()
    return nc


def kernel(input1, input2, weight, bias, type_index):
    global _cached_nc, LAST_RESULTS

    input1 = np.asarray(input1, dtype=np.float32)
    input2 = np.asarray(input2, dtype=np.float32)
    weight = np.asarray(weight, dtype=np.float32)
    bias = np.asarray(bias, dtype=np.float32).reshape(-1)
    w_r = weight[int(type_index)]  # [D]

    # Host-side prep: fold the w_r row-scale into input1, split k-columns
    # by |w_r| (largest -> fp16, smallest -> fp8), lay both GEMM operands
    # out K-major (device accumulates in fp32).
    order = np.argsort(-np.abs(w_r), kind="stable")
    hi = np.sort(order[:DH])
    lo = np.sort(order[DH:])
    f8 = ml_dtypes.float8_e4m3
    scaled = input1 * w_r[None, :]  # [N1, D]
    rhsH = np.ascontiguousarray(input2[:, hi].T).astype(np.float16)  # [DH, N2]
    rhsL = np.ascontiguousarray(input2[:, lo].T).astype(f8)          # [DL, N2]
    bias_vec = np.full((P, 1), float(bias[0]), dtype=np.float32)

    scaledH = scaled[:, hi]
    scaledL = scaled[:, lo]
    in_maps = []
    for c in range(N_CORES):
        sl = slice(c * M, (c + 1) * M)
        in_maps.append(
            {
                "lhsTH": np.ascontiguousarray(scaledH[sl].T).astype(np.float16),
                "lhsTL": np.ascontiguousarray(scaledL[sl].T).astype(f8),
                "rhsH": rhsH,
                "rhsL": rhsL,
                "biasv": bias_vec,
            }
        )

    if _cached_nc is None:
        _cached_nc = _build()

    res = run_bass_kernel_spmd(
        _cached_nc, in_maps, core_ids=list(range(N_CORES)), trace=TRACE
    )
    LAST_RESULTS = res
    return np.concatenate(
        [res.results[c]["out"] for c in range(N_CORES)], axis=0
    ).astype(np.float32)
